# revision 9
# baseline (speedup 1.0000x reference)
"""TRN2 Bass kernel for CenterBBoxHead: conv head on 8 NeuronCores (data-parallel
over batch x H-half), decode + circle-NMS.

Sharding: core c -> image c//2, H-half c%2. Each core computes the shared conv
(384->64, 3x3, BN+ReLU) and the 5 branch heads for its 128-row half with enough
halo (host-padded x slice of 134 rows) that no inter-core communication is
needed for the dense maps.

Precision: the top-500 selection operates on sigmoid scores deep in saturation
(gaps < 1e-6), so the hm chain (shared conv -> hm branch) and the center branch
(NMS distance margins ~2e-3) are computed in compensated fp32r
(x*w ~= xr*wr + xr*wlr + xlr*wr, each fp32r matmul full-rate at N>=256), giving
fp32-class accuracy at ~3x fp32r cost instead of 4x for native fp32. The
rot/dim/center_z branches are plain fp32r (2e-4 rel, value-only outputs).
"""
import sys
import os

sys.path.insert(0, '/opt/trn_rl_repo')
import numpy as np

from concourse import bass, bacc, mybir, bass_utils
from concourse.tile import TileContext

F32 = mybir.dt.float32
F32R = mybir.dt.float32r
AF = mybir.ActivationFunctionType

B, CIN, H, W = 4, 384, 256, 256
CSH = 64
NUM_CLS = 3
K = 500
STRIDE = 2
VX = VY = 0.32
PC0 = PC1 = -75.2
SCORE_THRESH = 0.1
NMS_DIST2 = 4.0
POST_LIMIT = np.array([-80.0, -80.0, -10.0, 80.0, 80.0, 10.0], np.float32)
BRANCHES = [('center', 2), ('center_z', 1), ('dim', 3), ('rot', 2), ('hm', 3)]
COMP = {'center': True, 'center_z': False, 'dim': False, 'rot': False, 'hm': True}
FIN_OFF = {'center': 0, 'center_z': 32, 'dim': 64, 'rot': 96}

NCORES = 8
TRACE = False          # set True (e.g. from test.py) to capture a HW profile
LAST_EXEC_NS = None
XROWS = 134            # r0-3 .. r1+3 (host zero-padded)
NPAIR_SH = 66          # F_0..F_131
NOUT_PAIRS = 64        # OUT_0..OUT_127


def _build_program(nout_pairs=NOUT_PAIRS):
    npair_sh = nout_pairs + 2
    nc = bacc.Bacc("TRN2", target_bir_lowering=False, debug=False, num_devices=NCORES)

    xh = nc.dram_tensor("xh", [CIN, XROWS, W], F32, kind="ExternalInput")
    wshp = nc.dram_tensor("wshp", [128, 27 * 64], F32, kind="ExternalInput")
    w1p = nc.dram_tensor("w1p", [64, 45 * 64], F32, kind="ExternalInput")
    w2p = nc.dram_tensor("w2p", [64, 45 * 4], F32, kind="ExternalInput")
    bns = nc.dram_tensor("bns", [6, 64], F32, kind="ExternalInput")
    bnb = nc.dram_tensor("bnb", [6, 64], F32, kind="ExternalInput")
    b2p = nc.dram_tensor("b2p", [5, 4], F32, kind="ExternalInput")
    mF = nc.dram_tensor("mF", [2, 516], F32, kind="ExternalInput")
    mH = nc.dram_tensor("mH", [2, 516], F32, kind="ExternalInput")

    outs = {}
    for name, cout in BRANCHES:
        outs[name] = nc.dram_tensor(f"o_{name}", [cout, 2 * nout_pairs, W], F32,
                                    kind="ExternalOutput")

    with TileContext(nc) as tc:
        with tc.tile_pool(name="wpool", bufs=1) as wp, \
             tc.tile_pool(name="ring", bufs=1) as rp, \
             tc.tile_pool(name="stage", bufs=2) as sp, \
             tc.tile_pool(name="psum", bufs=1, space="PSUM") as pp, \
             tc.tile_pool(name="psum2", bufs=2, space="PSUM") as pp2:

            # ---- weight prep: load, round to fp32r, split residuals ----
            wsh32 = wp.tile([128, 27 * 64], F32)
            nc.sync.dma_start(wsh32[:], wshp[:])
            wshr = wp.tile([128, 27 * 64], F32R)
            wshl = wp.tile([128, 27 * 64], F32R)
            nc.vector.tensor_copy(wshr[:], wsh32[:])
            nc.vector.tensor_sub(wshl[:], wsh32[:], wshr[:].bitcast(F32))

            w132 = wp.tile([64, 45 * 64], F32)
            nc.sync.dma_start(w132[:], w1p[:])
            w1r = wp.tile([64, 45 * 64], F32R)
            w1l = wp.tile([64, 45 * 64], F32R)
            nc.vector.tensor_copy(w1r[:], w132[:])
            nc.vector.tensor_sub(w1l[:], w132[:], w1r[:].bitcast(F32))

            w232 = wp.tile([64, 45 * 4], F32)
            nc.sync.dma_start(w232[:], w2p[:])
            w2r = wp.tile([64, 45 * 4], F32R)
            w2l = wp.tile([64, 45 * 4], F32R)
            nc.vector.tensor_copy(w2r[:], w232[:])
            nc.vector.tensor_sub(w2l[:], w232[:], w2r[:].bitcast(F32))

            def wsh_ap(t, k, which):
                j = (k * 9 + t) * 64
                return {'r': wshr, 'l': wshl}[which][:, j:j + 64]

            def w1_ap(b, t, which):
                j = (b * 9 + t) * 64
                return {'r': w1r, 'l': w1l}[which][:, j:j + 64]

            def w2_ap(b, t, cout, which):
                j = (b * 9 + t) * 4
                return {'r': w2r, 'l': w2l}[which][:, j:j + cout]

            bn_s = {}
            bn_b = {}
            for i, nm in enumerate(['shared'] + [b for b, _ in BRANCHES]):
                ts_ = wp.tile([64, 1], F32, tag=f"bns{i}", name=f"bns{i}")
                tb_ = wp.tile([64, 1], F32, tag=f"bnb{i}", name=f"bnb{i}")
                nc.sync.dma_start(ts_[:], bns[i].rearrange("(k o) -> k o", o=1))
                nc.sync.dma_start(tb_[:], bnb[i].rearrange("(k o) -> k o", o=1))
                bn_s[nm] = ts_
                bn_b[nm] = tb_
            b2_t = {}
            for bi, (nm, cout) in enumerate(BRANCHES):
                bt = wp.tile([cout, 1], F32, tag=f"b2_{nm}", name=f"b2_{nm}")
                nc.sync.dma_start(bt[:], b2p[bi, :cout].rearrange("(k o) -> k o", o=1))
                b2_t[nm] = bt

            # ---- rings (4 rows each, 258 wide with zero pad cols 0 and 257) ----
            WD = 258
            xe = [rp.tile([128, 4, WD], F32, tag=f"xe{c}", name=f"xe{c}") for c in range(3)]
            xr = [rp.tile([128, 4, WD], F32R, tag=f"xr{c}", name=f"xr{c}") for c in range(3)]
            xl = [rp.tile([128, 4, WD], F32R, tag=f"xl{c}", name=f"xl{c}") for c in range(3)]
            fe = rp.tile([64, 4, WD], F32)
            fr = rp.tile([64, 4, WD], F32R)
            fl = rp.tile([64, 4, WD], F32R)
            hr = {}
            he = {}
            hl = {}
            for nm, _ in BRANCHES:
                hr[nm] = rp.tile([64, 4, WD], F32R, tag=f"hr_{nm}", name=f"hr_{nm}")
                if COMP[nm]:
                    he[nm] = rp.tile([64, 4, WD], F32, tag=f"he_{nm}", name=f"he_{nm}")
                    hl[nm] = rp.tile([64, 4, WD], F32R, tag=f"hl_{nm}", name=f"hl_{nm}")
            # zero-init all rings (pad cols 0 and 257 stay zero forever; data
            # cols get overwritten). fp32r rings must be produced by compute
            # ops, so copy/sub from a zeroed fp32 tile.
            for t in xe + [fe] + list(he.values()):
                nc.vector.memset(t[:], 0.0)
            zz = rp.tile([128, 4, WD], F32)
            nc.vector.memset(zz[:], 0.0)
            for t in xr + xl:
                nc.vector.tensor_copy(t[:], zz[:])
            for t in [fr, fl] + list(hr.values()) + list(hl.values()):
                nc.vector.tensor_copy(t[:], zz[:64])

            mFf = rp.tile([64, 516], F32)
            mFl = rp.tile([64, 516], F32)
            mHf = rp.tile([64, 516], F32)
            mHl = rp.tile([64, 516], F32)
            for tdst, tsrc, row in ((mFf, mF, 0), (mFl, mF, 1), (mHf, mH, 0), (mHl, mH, 1)):
                tmp1 = rp.tile([1, 516], F32, tag="m1tmp", name="m1tmp")
                nc.sync.dma_start(tmp1[:], tsrc[row].rearrange("(o n) -> o n", o=1))
                nc.gpsimd.partition_broadcast(tdst[:], tmp1[:], channels=64)

            def load_x_row(j):
                pos = j % 4
                for c in range(3):
                    nc.sync.dma_start(xe[c][:, pos, 1:257], xh[c * 128:(c + 1) * 128, j, :])
                    nc.vector.tensor_copy(xr[c][:, pos, 1:257], xe[c][:, pos, 1:257])
                    nc.vector.tensor_sub(xl[c][:, pos, 1:257], xe[c][:, pos, 1:257],
                                         xr[c][:, pos, 1:257].bitcast(F32))

            def flush_group(group):
                n = len(group)
                for i, (ps_ap, lhsT, rhs, tp) in enumerate(group):
                    nc.tensor.matmul(ps_ap, lhsT, rhs, start=(i == 0), stop=(i == n - 1),
                                     tile_position=tp)
                group.clear()

            def add_rows(group, ps, po, cout, lhsT, ring, row0, dx, tp=None):
                p0 = row0 % 4
                if p0 != 3:
                    group.append((ps[po:po + cout, 0:512], lhsT,
                                  ring[:, p0:p0 + 2, dx:dx + 256], tp))
                else:
                    for r in range(2):
                        p = (row0 + r) % 4
                        group.append((ps[po:po + cout, r * 256:(r + 1) * 256], lhsT,
                                      ring[:, p, dx:dx + 256], tp))

            # ---- main pipeline ----
            for j in range(4):
                load_x_row(j)

            for s in range(npair_sh):
                ps_sh = pp.tile([64, 512], F32, tag="ps_sh")
                g = []
                for k in range(3):
                    for dy in range(3):
                        for dx in range(3):
                            add_rows(g, ps_sh, 0, 64, wsh_ap(dy * 3 + dx, k, 'r'),
                                     xr[k], 2 * s + dy, dx)
                for k in range(3):
                    for dy in range(3):
                        for dx in range(3):
                            t = dy * 3 + dx
                            add_rows(g, ps_sh, 0, 64, wsh_ap(t, k, 'l'), xr[k], 2 * s + dy, dx)
                            add_rows(g, ps_sh, 0, 64, wsh_ap(t, k, 'r'), xl[k], 2 * s + dy, dx)
                flush_group(g)
                fpos = (2 * s) % 4
                nc.scalar.activation(fe[:, fpos:fpos + 2, 1:257],
                                     ps_sh[:].rearrange("p (r c) -> p r c", r=2),
                                     AF.Relu, bias=bn_b['shared'][:], scale=bn_s['shared'][:])
                if s == 0 or s == npair_sh - 1:
                    mm = mFf if s == 0 else mFl
                    fv = fe[:, fpos:fpos + 2, :].rearrange("p r c -> p (r c)")
                    nc.vector.tensor_mul(fv, fv, mm[:])
                nc.vector.tensor_copy(fr[:, fpos:fpos + 2, :], fe[:, fpos:fpos + 2, :])
                nc.vector.tensor_sub(fl[:, fpos:fpos + 2, :], fe[:, fpos:fpos + 2, :],
                                     fr[:, fpos:fpos + 2, :].bitcast(F32))

                if s + 1 < npair_sh:
                    load_x_row(2 * s + 4)
                    load_x_row(2 * s + 5)

                if s >= 1:
                    t_ = s - 1
                    for bi, (nm, cout) in enumerate(BRANCHES):
                        ps1 = pp2.tile([64, 512], F32, tag="ps1", name="ps1")
                        g = []
                        for dy in range(3):
                            for dx in range(3):
                                add_rows(g, ps1, 0, 64, w1_ap(bi, dy * 3 + dx, 'r'),
                                         fr, 2 * t_ + dy, dx)
                        if COMP[nm]:
                            for dy in range(3):
                                for dx in range(3):
                                    tt = dy * 3 + dx
                                    add_rows(g, ps1, 0, 64, w1_ap(bi, tt, 'l'), fr, 2 * t_ + dy, dx)
                                    add_rows(g, ps1, 0, 64, w1_ap(bi, tt, 'r'), fl, 2 * t_ + dy, dx)
                        flush_group(g)
                        hpos = (2 * t_) % 4
                        hmask = None
                        if t_ == 0:
                            hmask = mHf
                        elif t_ == npair_sh - 2:
                            hmask = mHl
                        if COMP[nm]:
                            nc.scalar.activation(he[nm][:, hpos:hpos + 2, 1:257],
                                                 ps1[:].rearrange("p (r c) -> p r c", r=2),
                                                 AF.Relu, bias=bn_b[nm][:], scale=bn_s[nm][:])
                            if hmask is not None:
                                hv = he[nm][:, hpos:hpos + 2, :].rearrange("p r c -> p (r c)")
                                nc.vector.tensor_mul(hv, hv, hmask[:])
                            nc.vector.tensor_copy(hr[nm][:, hpos:hpos + 2, :],
                                                  he[nm][:, hpos:hpos + 2, :])
                            nc.vector.tensor_sub(hl[nm][:, hpos:hpos + 2, :],
                                                 he[nm][:, hpos:hpos + 2, :],
                                                 hr[nm][:, hpos:hpos + 2, :].bitcast(F32))
                        else:
                            nc.scalar.activation(hr[nm][:, hpos:hpos + 2, 1:257],
                                                 ps1[:].rearrange("p (r c) -> p r c", r=2),
                                                 AF.Relu, bias=bn_b[nm][:], scale=bn_s[nm][:])
                            if hmask is not None:
                                hv = hr[nm][:, hpos:hpos + 2, :].rearrange("p r c -> p (r c)")
                                nc.vector.tensor_mul(hv, hv.bitcast(F32), hmask[:])

                if s >= 2 and s - 2 < nout_pairs:
                    u = s - 2
                    for bi, (nm, cout) in enumerate(BRANCHES):
                        psf = pp.tile([cout, 512], F32, tag=f"psf_{nm}", name=f"psf_{nm}")
                        g = []
                        for dy in range(3):
                            for dx in range(3):
                                tt = dy * 3 + dx
                                row0 = 2 * u + dy
                                add_rows(g, psf, 0, cout, w2_ap(bi, tt, cout, 'r'),
                                         hr[nm], row0, dx)
                                if COMP[nm]:
                                    add_rows(g, psf, 0, cout, w2_ap(bi, tt, cout, 'l'),
                                             hr[nm], row0, dx)
                                    add_rows(g, psf, 0, cout, w2_ap(bi, tt, cout, 'r'),
                                             hl[nm], row0, dx)
                        flush_group(g)
                        stg = sp.tile([cout, 512], F32, tag=f"st_{nm}", name=f"st_{nm}")
                        nc.scalar.activation(stg[:], psf[:], AF.Identity, bias=b2_t[nm][:])
                        nc.sync.dma_start(
                            outs[nm][:, 2 * u:2 * u + 2, :],
                            stg[:].rearrange("p (r c) -> p r c", r=2))

    nc.compile()
    return nc


def _host_decode(center, center_z, dim, rot, hm):
    """Numpy mirror of the reference decode + circle NMS (full batch)."""
    Bq, C, Hq, Wq = hm.shape
    s = (1.0 / (1.0 + np.exp(-hm.astype(np.float32)))).astype(np.float32)
    rois = np.zeros((Bq, K, 7), np.float32)
    roi_scores = np.zeros((Bq, K), np.float32)
    roi_labels = np.zeros((Bq, K), np.int32)
    for b in range(Bq):
        sc = s[b].reshape(C, Hq * Wq)
        idx_c = np.argsort(-sc, axis=1, kind='stable')[:, :K]
        val_c = np.take_along_axis(sc, idx_c, axis=1)
        flat = val_c.reshape(-1)
        ind = np.argsort(-flat, kind='stable')[:K]
        scores = flat[ind]
        classes = (ind // K).astype(np.int32)
        inds = idx_c.reshape(-1)[ind]
        ys = (inds // Wq).astype(np.float32)
        xs = (inds % Wq).astype(np.float32)

        def gather(f):
            return f[b].reshape(f.shape[1], Hq * Wq)[:, inds].T

        c = gather(center)
        z = gather(center_z)[:, 0]
        d = np.exp(gather(dim)).astype(np.float32)
        r = gather(rot)
        ang = np.arctan2(r[:, 1], r[:, 0]).astype(np.float32)
        xw = ((xs + c[:, 0]) * STRIDE * VX + PC0).astype(np.float32)
        yw = ((ys + c[:, 1]) * STRIDE * VY + PC1).astype(np.float32)
        boxes = np.concatenate([xw[:, None], yw[:, None], z[:, None], d, ang[:, None]],
                               axis=-1).astype(np.float32)
        in_range = np.all((boxes[:, :3] >= POST_LIMIT[:3]) & (boxes[:, :3] <= POST_LIMIT[3:]),
                          axis=-1)
        valid = (scores > SCORE_THRESH) & in_range
        dxm = boxes[:, None, 0] - boxes[None, :, 0]
        dym = boxes[:, None, 1] - boxes[None, :, 1]
        sup = (dxm * dxm + dym * dym) < NMS_DIST2
        keep = valid.copy()
        for i in range(K):
            if keep[i]:
                keep[i + 1:] &= ~sup[i, i + 1:]
        rois[b] = boxes * keep[:, None]
        roi_scores[b] = scores * keep
        roi_labels[b] = np.where(keep, classes, 0)
    return rois, roi_scores, roi_labels


def _prep_weights(params):
    def getp(d, k):
        return np.asarray(d[k], np.float32)

    eps = 1e-5
    names = ['shared'] + [b for b, _ in BRANCHES]
    bns = np.zeros((6, 64), np.float32)
    bnb = np.zeros((6, 64), np.float32)
    for i, nm in enumerate(names):
        pb = params[nm]
        sc = getp(pb, 'gamma') / np.sqrt(getp(pb, 'var') + eps)
        bns[i] = sc
        bnb[i] = getp(pb, 'beta') - getp(pb, 'mean') * sc

    wsh_full = getp(params['shared'], 'w')  # [64, 384, 3, 3]
    wshp = np.zeros((128, 27 * 64), np.float32)
    for k in range(3):
        for dy in range(3):
            for dx in range(3):
                j = (k * 9 + dy * 3 + dx) * 64
                wshp[:, j:j + 64] = wsh_full[:, k * 128:(k + 1) * 128, dy, dx].T
    w1p = np.zeros((64, 45 * 64), np.float32)
    w2p = np.zeros((64, 45 * 4), np.float32)
    b2_l = np.zeros((5, 4), np.float32)
    for bi, (nm, cout) in enumerate(BRANCHES):
        pb = params[nm]
        w1f = getp(pb, 'w1')
        w2f = getp(pb, 'w2')
        for dy in range(3):
            for dx in range(3):
                t = dy * 3 + dx
                w1p[:, (bi * 9 + t) * 64:(bi * 9 + t) * 64 + 64] = w1f[:, :, dy, dx].T
                w2p[:, (bi * 9 + t) * 4:(bi * 9 + t) * 4 + cout] = w2f[:, :, dy, dx].T
        b2_l[bi, :cout] = getp(pb, 'b2')
    return dict(wshp=wshp, w1p=w1p, w2p=w2p, bns=bns, bnb=bnb, b2p=b2_l)


def _row_masks(core, nout_pairs=NOUT_PAIRS):
    npair_sh = nout_pairs + 2
    img, half = core // 2, core % 2
    r0 = half * 128
    mF = np.zeros((2, 516), np.float32)
    mH = np.zeros((2, 516), np.float32)
    for r in range(2):
        if 0 <= (r0 - 2 + r) <= H - 1:
            mF[0, r * 258:(r + 1) * 258] = 1.0
        if 0 <= (r0 - 2 + 2 * (npair_sh - 1) + r) <= H - 1:
            mF[1, r * 258:(r + 1) * 258] = 1.0
        if 0 <= (r0 - 1 + r) <= H - 1:
            mH[0, r * 258:(r + 1) * 258] = 1.0
        if 0 <= (r0 - 1 + 2 * (npair_sh - 2) + r) <= H - 1:
            mH[1, r * 258:(r + 1) * 258] = 1.0
    return mF, mH


def _x_slice(x, core):
    img, half = core // 2, core % 2
    r0 = half * 128
    xs = np.zeros((CIN, XROWS, W), np.float32)
    lo, hi = r0 - 3, r0 + 131
    src_lo, src_hi = max(lo, 0), min(hi, H)
    xs[:, src_lo - lo:src_hi - lo, :] = x[img, :, src_lo:src_hi, :]
    return xs


def kernel(x, params):
    x = np.asarray(x, np.float32)
    wd = _prep_weights(params)
    nc = _build_program()
    in_maps = []
    for c in range(NCORES):
        mFv, mHv = _row_masks(c)
        m = {"xh": _x_slice(x, c), "mF": mFv, "mH": mHv}
        m.update(wd)
        in_maps.append(m)

    global LAST_EXEC_NS
    res = bass_utils.run_bass_kernel_spmd(nc, in_maps, core_ids=list(range(NCORES)),
                                          trace=TRACE)
    LAST_EXEC_NS = getattr(res, 'exec_time_ns', None)

    full = {}
    for nm, cout in BRANCHES:
        full[nm] = np.zeros((B, cout, H, W), np.float32)
    for c in range(NCORES):
        img, half = c // 2, c % 2
        r0 = half * 128
        for nm, cout in BRANCHES:
            full[nm][img, :, r0:r0 + 128, :] = res.results[c][f"o_{nm}"]

    rois, roi_scores, roi_labels = _host_decode(
        full['center'], full['center_z'], full['dim'], full['rot'], full['hm'])
    return (full['center'], full['center_z'], full['dim'], full['rot'], full['hm'],
            rois, roi_scores, roi_labels)


# revision 11
# speedup vs baseline: 1.1688x; 1.1688x over previous
"""TRN2 Bass kernel for CenterBBoxHead: conv head on 8 NeuronCores (data-parallel
over batch x H-half), decode + circle-NMS.

Sharding: core c -> image c//2, H-half c%2. Each core computes the shared conv
(384->64, 3x3, BN+ReLU) and the 5 branch heads for its 128-row half with enough
halo (host-padded x slice of 134 rows) that no inter-core communication is
needed for the dense maps.

Precision: the top-500 selection operates on sigmoid scores deep in saturation
(gaps < 1e-6), so the hm chain (shared conv -> hm branch) and the center branch
(NMS distance margins ~2e-3) are computed in compensated fp32r
(x*w ~= xr*wr + xr*wlr + xlr*wr, each fp32r matmul full-rate at N>=256), giving
fp32-class accuracy at ~3x fp32r cost instead of 4x for native fp32. The
rot/dim/center_z branches are plain fp32r (2e-4 rel, value-only outputs).
"""
import sys
import os

sys.path.insert(0, '/opt/trn_rl_repo')
import numpy as np

from concourse import bass, bacc, mybir, bass_utils
from concourse.tile import TileContext

F32 = mybir.dt.float32
F32R = mybir.dt.float32r
AF = mybir.ActivationFunctionType

B, CIN, H, W = 4, 384, 256, 256
CSH = 64
NUM_CLS = 3
K = 500
STRIDE = 2
VX = VY = 0.32
PC0 = PC1 = -75.2
SCORE_THRESH = 0.1
NMS_DIST2 = 4.0
POST_LIMIT = np.array([-80.0, -80.0, -10.0, 80.0, 80.0, 10.0], np.float32)
BRANCHES = [('center', 2), ('center_z', 1), ('dim', 3), ('rot', 2), ('hm', 3)]
COMP = {'center': True, 'center_z': False, 'dim': False, 'rot': False, 'hm': True}
FIN_OFF = {'center': 0, 'center_z': 32, 'dim': 64, 'rot': 96}

NCORES = 8
TRACE = False          # set True (e.g. from test.py) to capture a HW profile
LAST_EXEC_NS = None
XROWS = 134            # r0-3 .. r1+3 (host zero-padded)
NPAIR_SH = 66          # F_0..F_131
NOUT_PAIRS = 64        # OUT_0..OUT_127


def _build_program(nout_pairs=NOUT_PAIRS):
    npair_sh = nout_pairs + 2
    nc = bacc.Bacc("TRN2", target_bir_lowering=False, debug=False, num_devices=NCORES)

    xh = nc.dram_tensor("xh", [CIN, XROWS, W], F32, kind="ExternalInput")
    wshp = nc.dram_tensor("wshp", [128, 27 * 64], F32, kind="ExternalInput")
    w1p = nc.dram_tensor("w1p", [64, 45 * 64], F32, kind="ExternalInput")
    w2p = nc.dram_tensor("w2p", [64, 45 * 4], F32, kind="ExternalInput")
    bns = nc.dram_tensor("bns", [6, 64], F32, kind="ExternalInput")
    bnb = nc.dram_tensor("bnb", [6, 64], F32, kind="ExternalInput")
    b2p = nc.dram_tensor("b2p", [5, 4], F32, kind="ExternalInput")
    w1q = nc.dram_tensor("w1q", [128, 15 * 64], F32, kind="ExternalInput")
    w2q = nc.dram_tensor("w2q", [128, 15 * 4], F32, kind="ExternalInput")
    mF = nc.dram_tensor("mF", [2, 516], F32, kind="ExternalInput")
    mH = nc.dram_tensor("mH", [2, 516], F32, kind="ExternalInput")

    outs = {}
    for name, cout in BRANCHES:
        outs[name] = nc.dram_tensor(f"o_{name}", [cout, 2 * nout_pairs, W], F32,
                                    kind="ExternalOutput")

    with TileContext(nc) as tc:
        with tc.tile_pool(name="wpool", bufs=1) as wp, \
             tc.tile_pool(name="ring", bufs=1) as rp, \
             tc.tile_pool(name="stage", bufs=2) as sp, \
             tc.tile_pool(name="psum", bufs=1, space="PSUM") as pp, \
             tc.tile_pool(name="psum2", bufs=2, space="PSUM") as pp2:

            # ---- weight prep: load, round to fp32r, split residuals ----
            wsh32 = wp.tile([128, 27 * 64], F32)
            nc.sync.dma_start(wsh32[:], wshp[:])
            wshr = wp.tile([128, 27 * 64], F32R)
            wshl = wp.tile([128, 27 * 64], F32R)
            nc.vector.tensor_copy(wshr[:], wsh32[:])
            nc.vector.tensor_sub(wshl[:], wsh32[:], wshr[:].bitcast(F32))

            w132 = wp.tile([64, 45 * 64], F32)
            nc.sync.dma_start(w132[:], w1p[:])
            w1r = wp.tile([64, 45 * 64], F32R)
            w1l = wp.tile([64, 45 * 64], F32R)
            nc.vector.tensor_copy(w1r[:], w132[:])
            nc.vector.tensor_sub(w1l[:], w132[:], w1r[:].bitcast(F32))

            w1q32 = wp.tile([128, 15 * 64], F32)
            nc.sync.dma_start(w1q32[:], w1q[:])
            w1qr = wp.tile([128, 15 * 64], F32R)
            w1ql = wp.tile([128, 15 * 64], F32R)
            nc.vector.tensor_copy(w1qr[:], w1q32[:])
            nc.vector.tensor_sub(w1ql[:], w1q32[:], w1qr[:].bitcast(F32))

            w2q32 = wp.tile([128, 15 * 4], F32)
            nc.sync.dma_start(w2q32[:], w2q[:])
            w2qr = wp.tile([128, 15 * 4], F32R)
            w2ql = wp.tile([128, 15 * 4], F32R)
            nc.vector.tensor_copy(w2qr[:], w2q32[:])
            nc.vector.tensor_sub(w2ql[:], w2q32[:], w2qr[:].bitcast(F32))

            w232 = wp.tile([64, 45 * 4], F32)
            nc.sync.dma_start(w232[:], w2p[:])
            w2r = wp.tile([64, 45 * 4], F32R)
            w2l = wp.tile([64, 45 * 4], F32R)
            nc.vector.tensor_copy(w2r[:], w232[:])
            nc.vector.tensor_sub(w2l[:], w232[:], w2r[:].bitcast(F32))

            def wsh_ap(t, k, which):
                j = (k * 9 + t) * 64
                return {'r': wshr, 'l': wshl}[which][:, j:j + 64]

            def w1_ap(b, t, which):
                j = (b * 9 + t) * 64
                return {'r': w1r, 'l': w1l}[which][:, j:j + 64]

            def w2_ap(b, t, cout, which):
                j = (b * 9 + t) * 4
                return {'r': w2r, 'l': w2l}[which][:, j:j + cout]

            def w1q_ap(b, dx, which):
                j = (b * 3 + dx) * 64
                return {'r': w1qr, 'l': w1ql}[which][:, j:j + 64]

            def w2q_ap(b, dx, cout, which):
                j = (b * 3 + dx) * 4
                return {'r': w2qr, 'l': w2ql}[which][:, j:j + cout]

            bn_s = {}
            bn_b = {}
            for i, nm in enumerate(['shared'] + [b for b, _ in BRANCHES]):
                ts_ = wp.tile([64, 1], F32, tag=f"bns{i}", name=f"bns{i}")
                tb_ = wp.tile([64, 1], F32, tag=f"bnb{i}", name=f"bnb{i}")
                nc.sync.dma_start(ts_[:], bns[i].rearrange("(k o) -> k o", o=1))
                nc.sync.dma_start(tb_[:], bnb[i].rearrange("(k o) -> k o", o=1))
                bn_s[nm] = ts_
                bn_b[nm] = tb_
            b2_t = {}
            for bi, (nm, cout) in enumerate(BRANCHES):
                bt = wp.tile([cout, 1], F32, tag=f"b2_{nm}", name=f"b2_{nm}")
                nc.sync.dma_start(bt[:], b2p[bi, :cout].rearrange("(k o) -> k o", o=1))
                b2_t[nm] = bt

            # ---- rings (4 rows each, 258 wide with zero pad cols 0 and 257) ----
            WD = 258
            xe = [rp.tile([128, 4, WD], F32, tag=f"xe{c}", name=f"xe{c}") for c in range(3)]
            xr = [rp.tile([128, 4, WD], F32R, tag=f"xr{c}", name=f"xr{c}") for c in range(3)]
            xl = [rp.tile([128, 4, WD], F32R, tag=f"xl{c}", name=f"xl{c}") for c in range(3)]
            fe = rp.tile([64, 4, WD], F32)
            fr = rp.tile([128, 4, WD], F32R)
            fl = rp.tile([128, 4, WD], F32R)
            hr = {}
            he = {}
            hl = {}
            for nm, _ in BRANCHES:
                hr[nm] = rp.tile([128, 4, WD], F32R, tag=f"hr_{nm}", name=f"hr_{nm}")
                if COMP[nm]:
                    he[nm] = rp.tile([64, 4, WD], F32, tag=f"he_{nm}", name=f"he_{nm}")
                    hl[nm] = rp.tile([128, 4, WD], F32R, tag=f"hl_{nm}", name=f"hl_{nm}")
            # zero-init all rings (pad cols 0 and 257 stay zero forever; data
            # cols get overwritten). fp32r rings must be produced by compute
            # ops, so copy/sub from a zeroed fp32 tile.
            for t in xe + [fe] + list(he.values()):
                nc.vector.memset(t[:], 0.0)
            zz = rp.tile([128, 4, WD], F32)
            nc.vector.memset(zz[:], 0.0)
            for t in xr + xl:
                nc.vector.tensor_copy(t[:], zz[:])
            for t in [fr, fl] + list(hr.values()) + list(hl.values()):
                nc.vector.tensor_copy(t[:], zz[:])

            mFf = rp.tile([64, 516], F32)
            mFl = rp.tile([64, 516], F32)
            mHf = rp.tile([64, 516], F32)
            mHl = rp.tile([64, 516], F32)
            for tdst, tsrc, row in ((mFf, mF, 0), (mFl, mF, 1), (mHf, mH, 0), (mHl, mH, 1)):
                tmp1 = rp.tile([1, 516], F32, tag="m1tmp", name="m1tmp")
                nc.sync.dma_start(tmp1[:], tsrc[row].rearrange("(o n) -> o n", o=1))
                nc.gpsimd.partition_broadcast(tdst[:], tmp1[:], channels=64)

            def load_x_row(j):
                pos = j % 4
                for c in range(3):
                    nc.sync.dma_start(xe[c][:, pos, 1:257], xh[c * 128:(c + 1) * 128, j, :])
                    nc.vector.tensor_copy(xr[c][:, pos, 1:257], xe[c][:, pos, 1:257])
                    nc.vector.tensor_sub(xl[c][:, pos, 1:257], xe[c][:, pos, 1:257],
                                         xr[c][:, pos, 1:257].bitcast(F32))

            def flush_group(group):
                n = len(group)
                for i, (ps_ap, lhsT, rhs, tp) in enumerate(group):
                    nc.tensor.matmul(ps_ap, lhsT, rhs, start=(i == 0), stop=(i == n - 1),
                                     tile_position=tp)
                group.clear()

            def add_rows(group, ps, po, cout, lhsT, ring, row0, dx, tp=None):
                p0 = row0 % 4
                if p0 != 3:
                    group.append((ps[po:po + cout, 0:512], lhsT,
                                  ring[:, p0:p0 + 2, dx:dx + 256], tp))
                else:
                    for r in range(2):
                        p = (row0 + r) % 4
                        group.append((ps[po:po + cout, r * 256:(r + 1) * 256], lhsT,
                                      ring[:, p, dx:dx + 256], tp))

            def dup_shift(ring, pos):
                # bottom half holds "next row": dst[64:128, q] = row q+1.
                # new rows at (pos, pos+1) -> dst q = (pos-1)%4 and pos.
                if pos == 2:
                    nc.vector.tensor_copy(ring[64:128, 1:3, :], ring[0:64, 2:4, :].bitcast(F32))
                else:  # pos == 0: dst (3, 0) <- src (0, 1)
                    nc.vector.tensor_copy(ring[64:128, 3, :], ring[0:64, 0, :].bitcast(F32))
                    nc.vector.tensor_copy(ring[64:128, 0, :], ring[0:64, 1, :].bitcast(F32))

            # ---- main pipeline ----
            for j in range(4):
                load_x_row(j)

            for s in range(npair_sh):
                ps_sh = pp.tile([64, 512], F32, tag="ps_sh")
                g = []
                for k in range(3):
                    for dy in range(3):
                        for dx in range(3):
                            add_rows(g, ps_sh, 0, 64, wsh_ap(dy * 3 + dx, k, 'r'),
                                     xr[k], 2 * s + dy, dx)
                for k in range(3):
                    for dy in range(3):
                        for dx in range(3):
                            t = dy * 3 + dx
                            add_rows(g, ps_sh, 0, 64, wsh_ap(t, k, 'l'), xr[k], 2 * s + dy, dx)
                            add_rows(g, ps_sh, 0, 64, wsh_ap(t, k, 'r'), xl[k], 2 * s + dy, dx)
                flush_group(g)
                fpos = (2 * s) % 4
                nc.scalar.activation(fe[:, fpos:fpos + 2, 1:257],
                                     ps_sh[:].rearrange("p (r c) -> p r c", r=2),
                                     AF.Relu, bias=bn_b['shared'][:], scale=bn_s['shared'][:])
                if s == 0 or s == npair_sh - 1:
                    mm = mFf if s == 0 else mFl
                    fv = fe[:, fpos:fpos + 2, :].rearrange("p r c -> p (r c)")
                    nc.vector.tensor_mul(fv, fv, mm[:])
                nc.vector.tensor_copy(fr[0:64, fpos:fpos + 2, :], fe[:, fpos:fpos + 2, :])
                nc.vector.tensor_sub(fl[0:64, fpos:fpos + 2, :], fe[:, fpos:fpos + 2, :],
                                     fr[0:64, fpos:fpos + 2, :].bitcast(F32))
                dup_shift(fr, fpos)
                dup_shift(fl, fpos)

                if s + 1 < npair_sh:
                    load_x_row(2 * s + 4)
                    load_x_row(2 * s + 5)

                if s >= 1:
                    t_ = s - 1
                    for bi, (nm, cout) in enumerate(BRANCHES):
                        ps1 = pp2.tile([64, 512], F32, tag="ps1", name="ps1")
                        g = []
                        if nm == 'hm':
                            for dy in range(3):
                                for dx in range(3):
                                    add_rows(g, ps1, 0, 64, w1_ap(bi, dy * 3 + dx, 'r'),
                                             fr[0:64], 2 * t_ + dy, dx)
                            for dy in range(3):
                                for dx in range(3):
                                    tt = dy * 3 + dx
                                    add_rows(g, ps1, 0, 64, w1_ap(bi, tt, 'l'),
                                             fr[0:64], 2 * t_ + dy, dx)
                                    add_rows(g, ps1, 0, 64, w1_ap(bi, tt, 'r'),
                                             fl[0:64], 2 * t_ + dy, dx)
                        else:
                            passes = [('r', fr)]
                            if COMP[nm]:
                                passes += [('l', fr), ('rl', fl)]
                            for which, ringx in passes:
                                wq = 'r' if which == 'rl' else which
                                for dx in range(3):
                                    # dy 0+1 packed (K=128): one matmul per output row
                                    for r in range(2):
                                        p = (2 * t_ + r) % 4
                                        g.append((ps1[0:64, r * 256:(r + 1) * 256],
                                                  w1q_ap(bi, dx, wq),
                                                  ringx[:, p, dx:dx + 256], None))
                                    # dy 2 (K=64)
                                    add_rows(g, ps1, 0, 64,
                                             w1_ap(bi, 6 + dx, wq if wq != 'rl' else 'r'),
                                             ringx[0:64], 2 * t_ + 2, dx)
                        flush_group(g)
                        hpos = (2 * t_) % 4
                        hmask = None
                        if t_ == 0:
                            hmask = mHf
                        elif t_ == npair_sh - 2:
                            hmask = mHl
                        if COMP[nm]:
                            nc.scalar.activation(he[nm][:, hpos:hpos + 2, 1:257],
                                                 ps1[:].rearrange("p (r c) -> p r c", r=2),
                                                 AF.Relu, bias=bn_b[nm][:], scale=bn_s[nm][:])
                            if hmask is not None:
                                hv = he[nm][:, hpos:hpos + 2, :].rearrange("p r c -> p (r c)")
                                nc.vector.tensor_mul(hv, hv, hmask[:])
                            nc.vector.tensor_copy(hr[nm][0:64, hpos:hpos + 2, :],
                                                  he[nm][:, hpos:hpos + 2, :])
                            nc.vector.tensor_sub(hl[nm][0:64, hpos:hpos + 2, :],
                                                 he[nm][:, hpos:hpos + 2, :],
                                                 hr[nm][0:64, hpos:hpos + 2, :].bitcast(F32))
                            dup_shift(hr[nm], hpos)
                            dup_shift(hl[nm], hpos)
                        else:
                            nc.scalar.activation(hr[nm][0:64, hpos:hpos + 2, 1:257],
                                                 ps1[:].rearrange("p (r c) -> p r c", r=2),
                                                 AF.Relu, bias=bn_b[nm][:], scale=bn_s[nm][:])
                            if hmask is not None:
                                hv = hr[nm][0:64, hpos:hpos + 2, :].rearrange("p r c -> p (r c)")
                                nc.vector.tensor_mul(hv, hv.bitcast(F32), hmask[:])
                            dup_shift(hr[nm], hpos)

                if s >= 2 and s - 2 < nout_pairs:
                    u = s - 2
                    for bi, (nm, cout) in enumerate(BRANCHES):
                        psf = pp.tile([cout, 512], F32, tag=f"psf_{nm}", name=f"psf_{nm}")
                        g = []
                        if nm == 'hm':
                            for dy in range(3):
                                for dx in range(3):
                                    tt = dy * 3 + dx
                                    row0 = 2 * u + dy
                                    add_rows(g, psf, 0, cout, w2_ap(bi, tt, cout, 'r'),
                                             hr[nm][0:64], row0, dx)
                                    add_rows(g, psf, 0, cout, w2_ap(bi, tt, cout, 'l'),
                                             hr[nm][0:64], row0, dx)
                                    add_rows(g, psf, 0, cout, w2_ap(bi, tt, cout, 'r'),
                                             hl[nm][0:64], row0, dx)
                        else:
                            passes = [('r', hr[nm])]
                            if COMP[nm]:
                                passes += [('l', hr[nm]), ('rl', hl[nm])]
                            for which, ringx in passes:
                                wq = 'r' if which == 'rl' else which
                                for dx in range(3):
                                    for r in range(2):
                                        p = (2 * u + r) % 4
                                        g.append((psf[0:cout, r * 256:(r + 1) * 256],
                                                  w2q_ap(bi, dx, cout, wq),
                                                  ringx[:, p, dx:dx + 256], None))
                                    add_rows(g, psf, 0, cout, w2_ap(bi, 6 + dx, cout, wq),
                                             ringx[0:64], 2 * u + 2, dx)
                        flush_group(g)
                        stg = sp.tile([cout, 512], F32, tag=f"st_{nm}", name=f"st_{nm}")
                        nc.scalar.activation(stg[:], psf[:], AF.Identity, bias=b2_t[nm][:])
                        nc.sync.dma_start(
                            outs[nm][:, 2 * u:2 * u + 2, :],
                            stg[:].rearrange("p (r c) -> p r c", r=2))

    nc.compile()
    return nc


def _host_decode(center, center_z, dim, rot, hm):
    """Numpy mirror of the reference decode + circle NMS (full batch)."""
    Bq, C, Hq, Wq = hm.shape
    s = (1.0 / (1.0 + np.exp(-hm.astype(np.float32)))).astype(np.float32)
    rois = np.zeros((Bq, K, 7), np.float32)
    roi_scores = np.zeros((Bq, K), np.float32)
    roi_labels = np.zeros((Bq, K), np.int32)
    for b in range(Bq):
        sc = s[b].reshape(C, Hq * Wq)
        idx_c = np.argsort(-sc, axis=1, kind='stable')[:, :K]
        val_c = np.take_along_axis(sc, idx_c, axis=1)
        flat = val_c.reshape(-1)
        ind = np.argsort(-flat, kind='stable')[:K]
        scores = flat[ind]
        classes = (ind // K).astype(np.int32)
        inds = idx_c.reshape(-1)[ind]
        ys = (inds // Wq).astype(np.float32)
        xs = (inds % Wq).astype(np.float32)

        def gather(f):
            return f[b].reshape(f.shape[1], Hq * Wq)[:, inds].T

        c = gather(center)
        z = gather(center_z)[:, 0]
        d = np.exp(gather(dim)).astype(np.float32)
        r = gather(rot)
        ang = np.arctan2(r[:, 1], r[:, 0]).astype(np.float32)
        xw = ((xs + c[:, 0]) * STRIDE * VX + PC0).astype(np.float32)
        yw = ((ys + c[:, 1]) * STRIDE * VY + PC1).astype(np.float32)
        boxes = np.concatenate([xw[:, None], yw[:, None], z[:, None], d, ang[:, None]],
                               axis=-1).astype(np.float32)
        in_range = np.all((boxes[:, :3] >= POST_LIMIT[:3]) & (boxes[:, :3] <= POST_LIMIT[3:]),
                          axis=-1)
        valid = (scores > SCORE_THRESH) & in_range
        dxm = boxes[:, None, 0] - boxes[None, :, 0]
        dym = boxes[:, None, 1] - boxes[None, :, 1]
        sup = (dxm * dxm + dym * dym) < NMS_DIST2
        keep = valid.copy()
        for i in range(K):
            if keep[i]:
                keep[i + 1:] &= ~sup[i, i + 1:]
        rois[b] = boxes * keep[:, None]
        roi_scores[b] = scores * keep
        roi_labels[b] = np.where(keep, classes, 0)
    return rois, roi_scores, roi_labels


def _prep_weights(params):
    def getp(d, k):
        return np.asarray(d[k], np.float32)

    eps = 1e-5
    names = ['shared'] + [b for b, _ in BRANCHES]
    bns = np.zeros((6, 64), np.float32)
    bnb = np.zeros((6, 64), np.float32)
    for i, nm in enumerate(names):
        pb = params[nm]
        sc = getp(pb, 'gamma') / np.sqrt(getp(pb, 'var') + eps)
        bns[i] = sc
        bnb[i] = getp(pb, 'beta') - getp(pb, 'mean') * sc

    wsh_full = getp(params['shared'], 'w')  # [64, 384, 3, 3]
    wshp = np.zeros((128, 27 * 64), np.float32)
    for k in range(3):
        for dy in range(3):
            for dx in range(3):
                j = (k * 9 + dy * 3 + dx) * 64
                wshp[:, j:j + 64] = wsh_full[:, k * 128:(k + 1) * 128, dy, dx].T
    w1p = np.zeros((64, 45 * 64), np.float32)
    w2p = np.zeros((64, 45 * 4), np.float32)
    b2_l = np.zeros((5, 4), np.float32)
    for bi, (nm, cout) in enumerate(BRANCHES):
        pb = params[nm]
        w1f = getp(pb, 'w1')
        w2f = getp(pb, 'w2')
        for dy in range(3):
            for dx in range(3):
                t = dy * 3 + dx
                w1p[:, (bi * 9 + t) * 64:(bi * 9 + t) * 64 + 64] = w1f[:, :, dy, dx].T
                w2p[:, (bi * 9 + t) * 4:(bi * 9 + t) * 4 + cout] = w2f[:, :, dy, dx].T
        b2_l[bi, :cout] = getp(pb, 'b2')
    w1qp = np.zeros((128, 15 * 64), np.float32)
    w2qp = np.zeros((128, 15 * 4), np.float32)
    for bi, (nm, cout) in enumerate(BRANCHES):
        pb = params[nm]
        w1f = getp(pb, 'w1')
        w2f = getp(pb, 'w2')
        for dx in range(3):
            j = (bi * 3 + dx) * 64
            w1qp[0:64, j:j + 64] = w1f[:, :, 0, dx].T
            w1qp[64:128, j:j + 64] = w1f[:, :, 1, dx].T
            j2 = (bi * 3 + dx) * 4
            w2qp[0:64, j2:j2 + cout] = w2f[:, :, 0, dx].T
            w2qp[64:128, j2:j2 + cout] = w2f[:, :, 1, dx].T
    return dict(wshp=wshp, w1p=w1p, w2p=w2p, w1q=w1qp, w2q=w2qp,
                bns=bns, bnb=bnb, b2p=b2_l)


def _row_masks(core, nout_pairs=NOUT_PAIRS):
    npair_sh = nout_pairs + 2
    img, half = core // 2, core % 2
    r0 = half * 128
    mF = np.zeros((2, 516), np.float32)
    mH = np.zeros((2, 516), np.float32)
    for r in range(2):
        if 0 <= (r0 - 2 + r) <= H - 1:
            mF[0, r * 258:(r + 1) * 258] = 1.0
        if 0 <= (r0 - 2 + 2 * (npair_sh - 1) + r) <= H - 1:
            mF[1, r * 258:(r + 1) * 258] = 1.0
        if 0 <= (r0 - 1 + r) <= H - 1:
            mH[0, r * 258:(r + 1) * 258] = 1.0
        if 0 <= (r0 - 1 + 2 * (npair_sh - 2) + r) <= H - 1:
            mH[1, r * 258:(r + 1) * 258] = 1.0
    return mF, mH


def _x_slice(x, core):
    img, half = core // 2, core % 2
    r0 = half * 128
    xs = np.zeros((CIN, XROWS, W), np.float32)
    lo, hi = r0 - 3, r0 + 131
    src_lo, src_hi = max(lo, 0), min(hi, H)
    xs[:, src_lo - lo:src_hi - lo, :] = x[img, :, src_lo:src_hi, :]
    return xs


def kernel(x, params):
    x = np.asarray(x, np.float32)
    wd = _prep_weights(params)
    nc = _build_program()
    in_maps = []
    for c in range(NCORES):
        mFv, mHv = _row_masks(c)
        m = {"xh": _x_slice(x, c), "mF": mFv, "mH": mHv}
        m.update(wd)
        in_maps.append(m)

    global LAST_EXEC_NS
    res = bass_utils.run_bass_kernel_spmd(nc, in_maps, core_ids=list(range(NCORES)),
                                          trace=TRACE)
    LAST_EXEC_NS = getattr(res, 'exec_time_ns', None)

    full = {}
    for nm, cout in BRANCHES:
        full[nm] = np.zeros((B, cout, H, W), np.float32)
    for c in range(NCORES):
        img, half = c // 2, c % 2
        r0 = half * 128
        for nm, cout in BRANCHES:
            full[nm][img, :, r0:r0 + 128, :] = res.results[c][f"o_{nm}"]

    rois, roi_scores, roi_labels = _host_decode(
        full['center'], full['center_z'], full['dim'], full['rot'], full['hm'])
    return (full['center'], full['center_z'], full['dim'], full['rot'], full['hm'],
            rois, roi_scores, roi_labels)
